# revision 50
# baseline (speedup 1.0000x reference)
"""Trainium2 Bass kernel for quantized (Q4_0) multi-head attention prefill.

Problem: nn_Attention_32023276159509
  B=1, S=2048, DIM=4096, 32 q-heads / 8 kv-heads (GQA x4), head_dim=128,
  Q4_0-packed int4 weights with per-64-group fp32 scales, RoPE (rotate-half),
  causal mask, softmax, output projection.

Sharding: tensor-parallel over heads across 8 NeuronCores. Core c owns
q-heads [4c, 4c+4), kv-head c, and wo input-columns [512c, 512(c+1)).
Each core computes a full [S, DIM] partial output; partials are summed on
the host (the all-reduce of the reference sharding recipe).

v3 design (vs v2 baseline, 607 us):
  - Phase-1 x stream: 1 MB 4-J-chunk DMAs (8 KB/partition descriptors per
    chunk via a rearranged AP) instead of 256 KB tile pairs. The per-DMA
    fixed cost amortizes: ~340 GB/s burst vs the ~133 GB/s that made
    phase 1 DMA-bound.
  - Weights in 8 grouped 1.5 MB loads on the sync queue.
  - D (softmax denominator) folded into the AV matmul: E is the stationary
    operand, V gets a ones column appended ([128,129] moving operand), so
    the unnormalized attention output lands as [q, hd | D] in PSUM. This
    kills 160 separate row-sum matmuls AND the [1,512] single-partition
    reciprocal (3.3 us each!) + partition_broadcast chain: the reciprocal
    is now a [128,1] per-partition op and the normalize is folded into the
    ACT-engine PSUM evacuation as a per-partition scale.
  - The [q, hd] result is transposed back to [hd, q] for the output
    projection with PE transposes that reuse the same PSUM bank in place.
  - Causal mask generated on-device with iota (0/1 bf16, multiplied into E
    post-exp) -- no mask DMA, no fp32 mask-add on the DVE.
  - wo prefetched at phase-2 open split across sync/scalar/gpsimd queues.
  - Output partials written bf16 as 16 x 1 MB HWDGE DMAs (was 128 x 256 KB
    SWDGE writes whose ~2.6 us serialized fixed costs bounded phase 2).
"""
import sys
import numpy as np

sys.path.insert(0, "/opt/trn_rl_repo")

import concourse.bass as bass  # noqa: E402
import concourse.tile as tile  # noqa: E402
from concourse import bacc, mybir, bass_utils  # noqa: E402
from contextlib import ExitStack  # noqa: E402
import ml_dtypes  # noqa: E402

F32 = mybir.dt.float32
F32R = mybir.dt.float32r
BF16 = mybir.dt.bfloat16
I32 = mybir.dt.int32
AOT = mybir.AluOpType
AFT = mybir.ActivationFunctionType

GROUP = 64
DIM = 4096
N_HEADS = 32
N_KV = 8
HEAD_DIM = 128
S = 2048
NCORES = 8
H_LOC = N_HEADS // NCORES          # 4 local q heads
QDIM_LOC = H_LOC * HEAD_DIM        # 512
SCALE = 1.0 / np.sqrt(np.float32(HEAD_DIM))
NEG = -1e9

QB = 512                            # q-block (seq columns per attention tile)
NQB = S // QB                       # 4
NKB = S // 128                      # 16 k-tiles of 128
VSP = 132                           # Vn_aug column stride per k-tile (129 used)
ASP = 132                           # psAV region stride (129 cols used)


def _build_kernel(causal: bool, dump: bool = False):
    """Build + compile the per-core Bass module. Same program on all cores."""
    nc = bacc.Bacc("TRN2", target_bir_lowering=False, debug=False)
    dbg = {}
    if dump:
        dbg["qt0"] = nc.dram_tensor("d_qt0", [128, S], F32, kind="ExternalOutput")
        dbg["kt"] = nc.dram_tensor("d_kt", [128, S], F32, kind="ExternalOutput")
        dbg["vt"] = nc.dram_tensor("d_vt", [128, S], BF16, kind="ExternalOutput")
        dbg["vn"] = nc.dram_tensor("d_vn", [128, NKB * VSP], BF16,
                                   kind="ExternalOutput")
        dbg["aot0"] = nc.dram_tensor("d_aot0", [128, S], BF16,
                                     kind="ExternalOutput")

    # ---- DRAM tensors (per-core inputs) ----
    # x and wqkv in bf16: the f32r LDWEIGHTS (224 ns) otherwise gates every
    # 216 ns phase-1 matmul; bf16 weights get FWL (~115-180 ns, hidden).
    xT_d = nc.dram_tensor("xT", [DIM, S], BF16, kind="ExternalInput")
    wqkvT_d = nc.dram_tensor("wqkvT", [DIM, 768], BF16, kind="ExternalInput")
    woT_d = nc.dram_tensor("woT", [QDIM_LOC, DIM], BF16, kind="ExternalInput")
    cosT_d = nc.dram_tensor("cosT", [128, S], F32, kind="ExternalInput")
    sinT_d = nc.dram_tensor("sinT", [128, S], F32, kind="ExternalInput")
    if not causal:
        mask_d = nc.dram_tensor("maskT", [S, S], F32, kind="ExternalInput")
    out_d = nc.dram_tensor("out_partial", [S, DIM], BF16, kind="ExternalOutput")

    with tile.TileContext(nc) as tc:
        with ExitStack() as top:
            # ---- persistent small constants ----
            cpool = top.enter_context(tc.tile_pool(name="const", bufs=1))
            iden_i = cpool.tile([128, 128], I32, tag="iden_i")
            nc.gpsimd.iota(iden_i[:], pattern=[[1, 128]], base=0,
                           channel_multiplier=-1)
            identb = cpool.tile([128, 128], BF16, tag="identb")
            nc.vector.tensor_scalar(identb[:], iden_i[:], 0, None, AOT.is_equal)

            # ---- persistent activations ----
            # All phase-2 SBUF pools are allocated TOP-LEVEL (before the
            # phase-1 pools): in stack order their space never overlaps the
            # phase-1 pools, so phase-2's first tiles don't wait for the
            # phase-1 release chain (which ends with qb3's rope + rot DMAs,
            # ~15us after the last phase-1 matmul). Only `stage` is entered
            # lazily at the first projection, by which time phase-1 space is
            # long free.
            qkv_pool = top.enter_context(tc.tile_pool(name="qkv", bufs=1))
            vt_pool = top.enter_context(tc.tile_pool(name="vt", bufs=1))
            aot_pool = top.enter_context(tc.tile_pool(name="aotp", bufs=1))
            wo_pool = top.enter_context(tc.tile_pool(name="wo", bufs=1))
            mk_pool = top.enter_context(tc.tile_pool(name="mk", bufs=1))
            e_pool = top.enter_context(tc.tile_pool(name="ep", bufs=6))
            at_pool = top.enter_context(tc.tile_pool(name="at", bufs=2))
            an_pool = top.enter_context(tc.tile_pool(name="an", bufs=2))
            QT = [qkv_pool.tile([128, S], F32R, tag=f"qt{h}", name=f"qt{h}")
                  for h in range(H_LOC)]
            KT = qkv_pool.tile([128, S], F32R, tag="kt")
            VT = vt_pool.tile([128, S], BF16, tag="vtt")   # V.T (bf16)
            # V natural + ones column, per k-tile regions of width VSP
            Vn = vt_pool.tile([128, NKB * VSP], BF16, tag="vn")
            nc.vector.memset(Vn[:], 1.0)   # ones col at kb*VSP+128 survives

            # =================== Phase 1: QKV projections + RoPE ===================
            with ExitStack() as p1:
                w_pool = p1.enter_context(tc.tile_pool(name="wqkv", bufs=1))
                trig = p1.enter_context(tc.tile_pool(name="trig", bufs=1))
                # bufs=3: with 2, the next qb's first chunk DMA serializes
                # behind this qb's J=24..27 matmuls (slot WAR) and the PE
                # then eats the full transfer+receipt latency (~6us per
                # qb boundary)
                xc_pool = p1.enter_context(tc.tile_pool(name="xc", bufs=3))
                rope_t = p1.enter_context(tc.tile_pool(name="rope", bufs=1))
                ps1 = p1.enter_context(tc.tile_pool(name="ps1", bufs=1, space="PSUM"))

                # W.T in 8 grouped loads (4 J-tiles each, 1.5 MB) on sync.
                WG = []
                for g in range(8):
                    wg = w_pool.tile([128, 4 * 768], BF16, tag=f"wg{g}", name=f"wg{g}")
                    if g == 0:
                        # split the first group in two so the very first
                        # matmuls (J0/J1) start ~2us earlier
                        for hh in range(2):
                            s2 = wqkvT_d.ap()[hh * 256:(hh + 1) * 256, :]
                            nc.sync.dma_start(
                                wg[:, hh * 1536:(hh + 1) * 1536].rearrange(
                                    "p (j c) -> p j c", j=2),
                                s2.rearrange("(j p) c -> p j c", j=2))
                    else:
                        src = wqkvT_d.ap()[g * 512:(g + 1) * 512, :]
                        nc.sync.dma_start(
                            wg[:].rearrange("p (j c) -> p j c", j=4),
                            src.rearrange("(j p) c -> p j c", j=4))
                    WG.append(wg)

                def Wsl(J, lo, hi):
                    g, j = J // 4, J % 4
                    return WG[g][:, j * 768 + lo:j * 768 + hi]

                # trig tiles; sign of rotate-half folded into sinTs rows
                # [0:64). On gpsimd: the HWDGE queues must deliver W/x first
                # (the first rope is ~30us in, gpsimd delivers these by ~15us)
                cosT = trig.tile([128, S], F32, tag="cosT")
                nc.gpsimd.dma_start(cosT[:], cosT_d.ap())
                sinTs = trig.tile([128, S], F32, tag="sinTs")
                nc.gpsimd.dma_start(sinTs[:], sinT_d.ap())
                nc.vector.tensor_scalar(sinTs[0:64, :], sinTs[0:64, :], -1.0, None,
                                        AOT.mult)

                for qb in range(NQB):
                    sl = slice(qb * QB, (qb + 1) * QB)
                    # x chunks: 8 per qb, 4 J-tiles each ([128, 2048] = 1 MB),
                    # alternating HWDGE queues.
                    xcs = []
                    for cidx in range(8):
                        xc = xc_pool.tile([128, 2048], BF16, tag="xc", name="xc")
                        src = xT_d.ap()[cidx * 512:(cidx + 1) * 512, sl]
                        src = src.rearrange("(j p) c -> p j c", j=4)
                        # qb0: keep x off the sync queue (it is serially
                        # draining the 34us of W-group loads); later qbs
                        # alternate queues.
                        if qb == 0:
                            eng = nc.scalar
                        else:
                            eng = nc.scalar if (qb * 8 + cidx) % 2 else nc.sync
                        eng.dma_start(xc[:].rearrange("p (j c) -> p j c", j=4),
                                      src)
                        xcs.append(xc)

                    # psq0/psq1 double-buffered: the next qb's first matmuls
                    # start in fresh banks while this qb's evacuations drain
                    psQ = [ps1.tile([128, QB], F32, tag=f"psq{h}", name=f"psq{h}",
                                    bufs=(2 if h < 2 else 1))
                           for h in range(H_LOC)]
                    psK = ps1.tile([128, QB], F32, tag="psk")
                    psV = ps1.tile([128, QB], F32, tag="psv")
                    for J in range(32):
                        xt = xcs[J // 4][:, (J % 4) * 512:(J % 4 + 1) * 512]
                        st, sp = (J == 0), (J == 31)
                        for h in range(H_LOC):
                            nc.tensor.matmul(psQ[h][:],
                                             Wsl(J, h * 128, (h + 1) * 128),
                                             xt, start=st, stop=sp)
                        nc.tensor.matmul(psK[:], Wsl(J, 512, 640), xt,
                                         start=st, stop=sp)
                        nc.tensor.matmul(psV[:], Wsl(J, 640, 768), xt,
                                         start=st, stop=sp)

                    # Evacuate all PSUM accumulators first (frees banks for
                    # the next qb's matmuls) on the ACT engine, so the DVE
                    # rope math below never blocks the PE's next qb.
                    # Single-buffered banks (psq2/psq3/psK/psV) evacuate
                    # first: they gate the next qb's first matmuls.
                    raws = [None] * H_LOC
                    for h in (2, 3):
                        raw = rope_t.tile([128, QB], F32, tag=f"raw{h}",
                                          name=f"raw{h}", bufs=1)
                        # DVE: runs in parallel with the ACT evacs below, so
                        # the single-buffered banks free ~2x faster
                        nc.vector.tensor_copy(raw[:], psQ[h][:])
                        raws[h] = raw
                    rawk = rope_t.tile([128, QB], F32, tag="rawk", name="rawk",
                                       bufs=1)
                    nc.scalar.activation(rawk[:], psK[:], AFT.Copy)
                    nc.scalar.activation(VT[:, sl], psV[:], AFT.Copy)
                    for h in (0, 1):
                        raw = rope_t.tile([128, QB], F32, tag=f"raw{h}",
                                          name=f"raw{h}", bufs=2)
                        nc.scalar.activation(raw[:], psQ[h][:], AFT.Copy)
                        raws[h] = raw

                    def rope_finish(raw, dst):
                        rot = rope_t.tile([128, QB], F32, tag="rot", name="rot",
                                          bufs=2)
                        # gpsimd issue queue: idle in phase 1, and NOT behind
                        # the weight/x chunk DMAs on the HWDGE queues
                        nc.gpsimd.dma_start(rot[0:64, :], raw[64:128, :])
                        nc.gpsimd.dma_start(rot[64:128, :], raw[0:64, :])
                        t1 = rope_t.tile([128, QB], F32, tag="t1", name="t1",
                                         bufs=2)
                        nc.vector.tensor_tensor(t1[:], raw[:], cosT[:, sl], AOT.mult)
                        nc.vector.tensor_tensor(rot[:], rot[:], sinTs[:, sl],
                                                AOT.mult)
                        nc.vector.tensor_tensor(dst[:, sl], t1[:], rot[:], AOT.add)

                    for h in range(H_LOC):
                        rope_finish(raws[h], QT[h])
                    rope_finish(rawk, KT)

            # =================== Phase 2: attention + output projection ===========
            AOT_t = [aot_pool.tile([128, S], BF16, tag=f"aot{h}", name=f"aot{h}")
                     for h in range(H_LOC)]

            with ExitStack() as p2:
                mkg_pool = p2.enter_context(tc.tile_pool(name="mkg", bufs=4))
                stage_holder = []
                ps_s = p2.enter_context(tc.tile_pool(name="ps_s", bufs=2, space="PSUM"))
                ps_av = p2.enter_context(tc.tile_pool(name="ps_av", bufs=2, space="PSUM"))
                ps_o = p2.enter_context(tc.tile_pool(name="ps_o", bufs=2, space="PSUM"))
                ps_t = p2.enter_context(tc.tile_pool(name="ps_t", bufs=1, space="PSUM"))

                # causal: 0/1 bf16 diagonal-tile masks generated on-device.
                # mask01[i][p, c] = 1.0 iff c - p >= i*128
                mtiles = []
                if causal:
                    iot = mk_pool.tile([128, QB], I32, tag="iot")
                    nc.gpsimd.iota(iot[:], pattern=[[1, QB]], base=0,
                                   channel_multiplier=-1)
                    for i in range(4):
                        mt = mk_pool.tile([128, QB], BF16, tag=f"mk{i}",
                                          name=f"mk{i}")
                        nc.vector.tensor_scalar(mt[:], iot[:], i * 128, None,
                                                AOT.is_ge)
                        mtiles.append(mt)

                # Wo (bf16, host-dequantized) resident: 4 head-tiles
                # [128, 4096], split across 3 queues so the first proj
                # (~15 us into phase 2) never waits.
                WOt = []
                for J in range(H_LOC):
                    wot = wo_pool.tile([128, DIM], BF16, tag=f"wo{J}", name=f"wo{J}")
                    eng = (nc.sync, nc.scalar, nc.gpsimd, nc.sync)[J]
                    eng.dma_start(wot[:], woT_d.ap()[J * 128:(J + 1) * 128, :])
                    WOt.append(wot)

                def proj_chunk(sb, feeder=None):
                    """Output projection + staged bf16 write for one 128-row
                    seq block. `feeder(1)` is called between ob groups so the
                    score->exp pipeline keeps filling while the PE runs proj
                    (otherwise ACT idles during proj and the PE starves on
                    exp afterwards)."""
                    ssl = slice(sb * 128, (sb + 1) * 128)
                    if not stage_holder:
                        stage_holder.append(
                            p2.enter_context(tc.tile_pool(name="stage", bufs=2)))
                    stg = stage_holder[0].tile([128, DIM], BF16, tag="stg",
                                               name="stg")
                    for ob in range(DIM // 512):
                        osl = slice(ob * 512, (ob + 1) * 512)
                        psO = ps_o.tile([128, 512], F32, tag="pso", name="pso")
                        for J in range(H_LOC):
                            nc.tensor.matmul(psO[:], AOT_t[J][:, ssl],
                                             WOt[J][:, osl],
                                             start=(J == 0), stop=(J == 3))
                        nc.vector.tensor_copy(stg[:, osl], psO[:])
                        if feeder is not None and ob % 2 == 1:
                            feeder(1)
                    # SWDGE: gpsimd is idle in phase 2 and this keeps the
                    # two HWDGE queues free. The last 4 chunks go HWDGE in
                    # two halves each: the first half (deps: obs 0-3 only)
                    # fires while obs 4-7 still project, shortening the
                    # kernel-tail drain.
                    if sb >= 4 * (NQB - 1):
                        eng = nc.scalar if sb % 2 else nc.sync
                        eng2 = nc.sync if sb % 2 else nc.scalar
                        half = DIM // 2
                        eng.dma_start(out_d.ap()[ssl, 0:half], stg[:, 0:half])
                        eng2.dma_start(out_d.ap()[ssl, half:], stg[:, half:])
                    else:
                        nc.gpsimd.dma_start(out_d.ap()[ssl, :], stg[:])

                def exp_tile(qb, h, kb):
                    sl = slice(qb * QB, (qb + 1) * QB)
                    psS = ps_s.tile([128, QB], F32, tag="pss", name="pss")
                    nc.tensor.matmul(psS[:],
                                     KT[:, kb * 128:(kb + 1) * 128],
                                     QT[h][:, sl], start=True, stop=True)
                    E = e_pool.tile([128, QB], BF16, tag="e", name="e")
                    nc.scalar.activation(E[:], psS[:], AFT.Exp,
                                         scale=float(SCALE))
                    if causal and kb >= 4 * qb:
                        nc.vector.tensor_tensor(E[:], E[:], mtiles[kb - 4 * qb][:],
                                                AOT.mult)
                    elif not causal:
                        mt = mkg_pool.tile([128, QB], F32, tag=f"mkg{kb % 4}",
                                           name=f"mkg{kb}")
                        nc.sync.dma_start(
                            mt[:], mask_d.ap()[kb * 128:(kb + 1) * 128, sl])
                        tmp = at_pool.tile([128, QB], F32, tag="sm", name="sm")
                        nc.vector.scalar_tensor_tensor(
                            tmp[:], psS[:], float(SCALE), mt[:],
                            AOT.mult, AOT.add)
                        nc.scalar.activation(E[:], tmp[:], AFT.Exp)
                    return E

                # Flat schedule over (qb, h, kb) with a global score/exp
                # pipeline LOOKAHEAD tiles deep -- the E queue stays primed
                # across head and q-block boundaries, so the PE's AV matmuls
                # never wait on the psS -> exp chain.
                flat = []
                for qb in range(NQB):
                    nkb = 4 * (qb + 1) if causal else NKB
                    for h in range(H_LOC):
                        for kb in range(nkb):
                            flat.append((qb, h, kb, nkb))
                LOOKAHEAD = 3
                EMAX = 5
                Equeue = []
                pidx = 0

                def feed(n):
                    # produce up to n lookahead score->exp tiles, bounded by
                    # the E pool depth
                    nonlocal pidx
                    emitted = 0
                    while (pidx < len(flat) and emitted < n
                           and len(Equeue) < EMAX):
                        pq, ph, pk, _ = flat[pidx]
                        Equeue.append(exp_tile(pq, ph, pk))
                        pidx += 1
                        emitted += 1

                def vt_transposes(qb):
                    # V.T -> V natural (bf16) on the PE (no DMA-xbar: its
                    # SBUF->SBUF-DMA hazard window opens under 8-core HBM
                    # contention), prefetched one q-block ahead.
                    for tkb in range(4 * qb, 4 * qb + 4):
                        pt = ps_t.tile([128, 128], BF16, tag="pt", name="pvt")
                        nc.tensor.transpose(
                            pt[:], VT[:, tkb * 128:(tkb + 1) * 128], identb[:])
                        nc.scalar.activation(
                            Vn[:, tkb * VSP:tkb * VSP + 128], pt[:], AFT.Copy)

                vt_transposes(0)
                feed(LOOKAHEAD + 1)
                psAV = [None, None]
                for idx, (qb, h, kb, nkb) in enumerate(flat):
                    if h == 0 and kb == 0 and qb + 1 < NQB:
                        vt_transposes(qb + 1)
                    if kb == 0:
                        # two banks, each holding two q-sub regions
                        # [q, hd | D] of width ASP
                        psAV[0] = ps_av.tile([128, 2 * ASP], F32, tag="av0",
                                             name="av0")
                        psAV[1] = ps_av.tile([128, 2 * ASP], F32, tag="av1",
                                             name="av1", bufs=1)
                    st, sp = (kb == 0), (kb == nkb - 1)
                    Ecur = Equeue.pop(0)
                    vsl = Vn[:, kb * VSP:kb * VSP + 129]
                    for qs in range(4):
                        # one accumulation group per PSUM bank: start marks
                        # the whole bank pending-zero, so only the first MM
                        # into the bank starts and only the last stops
                        nc.tensor.matmul(
                            psAV[qs // 2][:, (qs % 2) * ASP:(qs % 2) * ASP + 129],
                            Ecur[:, qs * 128:(qs + 1) * 128],
                            vsl,
                            start=(st and qs % 2 == 0),
                            stop=(sp and qs % 2 == 1))
                    if sp:
                        # normalize (reciprocal of the folded D column as a
                        # per-partition ACT scale) + DMA-xbar transpose back
                        # to [hd, q] per q-sub
                        rec = at_pool.tile([128, 4], F32, tag="rec", name="rec")
                        for qs in range(4):
                            reg = psAV[qs // 2][:, (qs % 2) * ASP:
                                                (qs % 2) * ASP + 129]
                            nc.vector.reciprocal(rec[:, qs:qs + 1],
                                                 reg[:, 128:129])
                            An = an_pool.tile([128, 128], BF16, tag="an",
                                              name="an")
                            # DVE: ACT is saturated by the exp stream
                            nc.vector.tensor_scalar(An[:], reg[:, 0:128],
                                                    rec[:, qs:qs + 1], None,
                                                    AOT.mult)
                            pt = ps_t.tile([128, 128], BF16, tag="pt",
                                           name="pat")
                            nc.tensor.transpose(pt[:], An[:], identb[:])
                            nc.scalar.activation(
                                AOT_t[h][:, qb * QB + qs * 128:
                                         qb * QB + (qs + 1) * 128],
                                pt[:], AFT.Copy)
                        # interleave the previous q-block's output projection:
                        # fills the normalize/transpose latency with PE work.
                        if qb > 0:
                            proj_chunk(4 * (qb - 1) + h, feeder=feed)
                    # maintain the lookahead AFTER consumption, so the next
                    # tiles' exps queue behind this tile's ACT work
                    while (pidx < len(flat) and pidx <= idx + 1 + LOOKAHEAD
                           and len(Equeue) < EMAX):
                        pq, ph, pk, _ = flat[pidx]
                        Equeue.append(exp_tile(pq, ph, pk))
                        pidx += 1
                for i in range(4):
                    proj_chunk(4 * (NQB - 1) + i)

                if dump:
                    nc.gpsimd.dma_start(dbg["qt0"].ap(), QT[0][:].bitcast(F32))
                    nc.gpsimd.dma_start(dbg["kt"].ap(), KT[:].bitcast(F32))
                    nc.gpsimd.dma_start(dbg["vt"].ap(), VT[:])
                    nc.gpsimd.dma_start(dbg["vn"].ap(), Vn[:])
                    nc.gpsimd.dma_start(dbg["aot0"].ap(), AOT_t[0][:])

    nc.compile()
    return nc


_BUILD_CACHE = {}


def _get_kernel(causal: bool):
    if causal not in _BUILD_CACHE:
        _BUILD_CACHE[causal] = _build_kernel(causal)
    return _BUILD_CACHE[causal]


def _dequant_np(packed, scales, out_f, in_f):
    """Exact numpy port of the reference dequantize_q40."""
    w = np.asarray(packed)
    if w.dtype != np.int8:
        w = w.astype(np.int8)
    s = np.asarray(scales, dtype=np.float32).reshape(-1, 1)
    msb = w >> 4                                   # arithmetic, sign-extends
    lsb = (w << 4) >> 4                            # int8 wraps, then sign-extends
    grp = np.concatenate([msb, lsb], axis=1).reshape(-1, GROUP).astype(np.float32)
    return (grp * s).reshape(out_f, in_f)


def _canonical_causal_mask():
    causal = np.triu(np.ones((S, S), dtype=bool), k=1)
    return np.where(causal, np.float32(NEG), np.float32(0.0)).astype(np.float32)


def build_in_maps(inputs, causal):
    x = np.asarray(inputs["x"], dtype=np.float32)
    cos = np.asarray(inputs["cos"], dtype=np.float32)
    sin = np.asarray(inputs["sin"], dtype=np.float32)

    xT = np.ascontiguousarray(x.reshape(S, DIM).T).astype(ml_dtypes.bfloat16)
    cosT = np.ascontiguousarray(np.concatenate([cos.T, cos.T], axis=0))  # [128, S]
    sinT = np.ascontiguousarray(np.concatenate([sin.T, sin.T], axis=0))

    Wq = _dequant_np(inputs["wq"], inputs["sq"], N_HEADS * HEAD_DIM, DIM)
    Wk = _dequant_np(inputs["wk"], inputs["sk"], N_KV * HEAD_DIM, DIM)
    Wv = _dequant_np(inputs["wv"], inputs["sv"], N_KV * HEAD_DIM, DIM)
    Wo = _dequant_np(inputs["wo"], inputs["so"], DIM, N_HEADS * HEAD_DIM)

    in_maps = []
    for c in range(NCORES):
        q0 = c * QDIM_LOC
        k0 = c * HEAD_DIM
        wqkvT = np.empty((DIM, 768), dtype=ml_dtypes.bfloat16)
        wqkvT[:, 0:512] = Wq[q0:q0 + QDIM_LOC].T.astype(ml_dtypes.bfloat16)
        wqkvT[:, 512:640] = Wk[k0:k0 + HEAD_DIM].T.astype(ml_dtypes.bfloat16)
        wqkvT[:, 640:768] = Wv[k0:k0 + HEAD_DIM].T.astype(ml_dtypes.bfloat16)
        woT = np.ascontiguousarray(
            Wo[:, q0:q0 + QDIM_LOC].T).astype(ml_dtypes.bfloat16)  # [512, 4096]
        m = dict(xT=xT, wqkvT=wqkvT, woT=woT, cosT=cosT, sinT=sinT)
        if not causal:
            mask = np.asarray(inputs["mask"], dtype=np.float32)
            m["maskT"] = np.ascontiguousarray(mask.T)
        in_maps.append(m)
    return in_maps


def kernel(**inputs):
    mask = np.asarray(inputs["mask"], dtype=np.float32)
    causal = bool(np.array_equal(mask, _canonical_causal_mask()))
    nc = _get_kernel(causal)
    in_maps = build_in_maps(inputs, causal)
    res = bass_utils.run_bass_kernel_spmd(nc, in_maps, core_ids=list(range(NCORES)))
    acc = np.zeros((S, DIM), dtype=np.float32)
    for r in res.results:
        acc += np.asarray(r["out_partial"]).astype(np.float32)
    return acc.reshape(1, S, DIM)


if __name__ == "__main__":
    print("building causal kernel...")
    _get_kernel(True)
    print("built")


# revision 51
# speedup vs baseline: 1.0099x; 1.0099x over previous
"""Trainium2 Bass kernel for quantized (Q4_0) multi-head attention prefill.

Problem: nn_Attention_32023276159509
  B=1, S=2048, DIM=4096, 32 q-heads / 8 kv-heads (GQA x4), head_dim=128,
  Q4_0-packed int4 weights with per-64-group fp32 scales, RoPE (rotate-half),
  causal mask, softmax, output projection.

Sharding: tensor-parallel over heads across 8 NeuronCores. Core c owns
q-heads [4c, 4c+4), kv-head c, and wo input-columns [512c, 512(c+1)).
Each core computes a full [S, DIM] partial output; partials are summed on
the host (the all-reduce of the reference sharding recipe).

v3 design (vs v2 baseline, 607 us):
  - Phase-1 x stream: 1 MB 4-J-chunk DMAs (8 KB/partition descriptors per
    chunk via a rearranged AP) instead of 256 KB tile pairs. The per-DMA
    fixed cost amortizes: ~340 GB/s burst vs the ~133 GB/s that made
    phase 1 DMA-bound.
  - Weights in 8 grouped 1.5 MB loads on the sync queue.
  - D (softmax denominator) folded into the AV matmul: E is the stationary
    operand, V gets a ones column appended ([128,129] moving operand), so
    the unnormalized attention output lands as [q, hd | D] in PSUM. This
    kills 160 separate row-sum matmuls AND the [1,512] single-partition
    reciprocal (3.3 us each!) + partition_broadcast chain: the reciprocal
    is now a [128,1] per-partition op and the normalize is folded into the
    ACT-engine PSUM evacuation as a per-partition scale.
  - The [q, hd] result is transposed back to [hd, q] for the output
    projection with PE transposes that reuse the same PSUM bank in place.
  - Causal mask generated on-device with iota (0/1 bf16, multiplied into E
    post-exp) -- no mask DMA, no fp32 mask-add on the DVE.
  - wo prefetched at phase-2 open split across sync/scalar/gpsimd queues.
  - Output partials written bf16 as 16 x 1 MB HWDGE DMAs (was 128 x 256 KB
    SWDGE writes whose ~2.6 us serialized fixed costs bounded phase 2).
"""
import sys
import numpy as np

sys.path.insert(0, "/opt/trn_rl_repo")

import concourse.bass as bass  # noqa: E402
import concourse.tile as tile  # noqa: E402
from concourse import bacc, mybir, bass_utils  # noqa: E402
from contextlib import ExitStack  # noqa: E402
import ml_dtypes  # noqa: E402

F32 = mybir.dt.float32
F32R = mybir.dt.float32r
BF16 = mybir.dt.bfloat16
I32 = mybir.dt.int32
AOT = mybir.AluOpType
AFT = mybir.ActivationFunctionType

GROUP = 64
DIM = 4096
N_HEADS = 32
N_KV = 8
HEAD_DIM = 128
S = 2048
NCORES = 8
H_LOC = N_HEADS // NCORES          # 4 local q heads
QDIM_LOC = H_LOC * HEAD_DIM        # 512
SCALE = 1.0 / np.sqrt(np.float32(HEAD_DIM))
NEG = -1e9

QB = 512                            # q-block (seq columns per attention tile)
NQB = S // QB                       # 4
NKB = S // 128                      # 16 k-tiles of 128
VSP = 132                           # Vn_aug column stride per k-tile (129 used)
ASP = 132                           # psAV region stride (129 cols used)


def _build_kernel(causal: bool, dump: bool = False):
    """Build + compile the per-core Bass module. Same program on all cores."""
    nc = bacc.Bacc("TRN2", target_bir_lowering=False, debug=False)
    dbg = {}
    if dump:
        dbg["qt0"] = nc.dram_tensor("d_qt0", [128, S], F32, kind="ExternalOutput")
        dbg["kt"] = nc.dram_tensor("d_kt", [128, S], F32, kind="ExternalOutput")
        dbg["vt"] = nc.dram_tensor("d_vt", [128, S], BF16, kind="ExternalOutput")
        dbg["vn"] = nc.dram_tensor("d_vn", [128, NKB * VSP], BF16,
                                   kind="ExternalOutput")
        dbg["aot0"] = nc.dram_tensor("d_aot0", [128, S], BF16,
                                     kind="ExternalOutput")

    # ---- DRAM tensors (per-core inputs) ----
    # x and wqkv in bf16: the f32r LDWEIGHTS (224 ns) otherwise gates every
    # 216 ns phase-1 matmul; bf16 weights get FWL (~115-180 ns, hidden).
    xT_d = nc.dram_tensor("xT", [DIM, S], BF16, kind="ExternalInput")
    wqkvT_d = nc.dram_tensor("wqkvT", [DIM, 768], BF16, kind="ExternalInput")
    woT_d = nc.dram_tensor("woT", [QDIM_LOC, DIM], BF16, kind="ExternalInput")
    cosT_d = nc.dram_tensor("cosT", [128, S], F32, kind="ExternalInput")
    sinT_d = nc.dram_tensor("sinT", [128, S], F32, kind="ExternalInput")
    if not causal:
        mask_d = nc.dram_tensor("maskT", [S, S], F32, kind="ExternalInput")
    out_d = nc.dram_tensor("out_partial", [S, DIM], BF16, kind="ExternalOutput")

    with tile.TileContext(nc) as tc:
        with ExitStack() as top:
            # ---- persistent small constants ----
            cpool = top.enter_context(tc.tile_pool(name="const", bufs=1))
            iden_i = cpool.tile([128, 128], I32, tag="iden_i")
            nc.gpsimd.iota(iden_i[:], pattern=[[1, 128]], base=0,
                           channel_multiplier=-1)
            identb = cpool.tile([128, 128], BF16, tag="identb")
            nc.vector.tensor_scalar(identb[:], iden_i[:], 0, None, AOT.is_equal)

            # ---- persistent activations ----
            # All phase-2 SBUF pools are allocated TOP-LEVEL (before the
            # phase-1 pools): in stack order their space never overlaps the
            # phase-1 pools, so phase-2's first tiles don't wait for the
            # phase-1 release chain (which ends with qb3's rope + rot DMAs,
            # ~15us after the last phase-1 matmul). Only `stage` is entered
            # lazily at the first projection, by which time phase-1 space is
            # long free.
            qkv_pool = top.enter_context(tc.tile_pool(name="qkv", bufs=1))
            vt_pool = top.enter_context(tc.tile_pool(name="vt", bufs=1))
            aot_pool = top.enter_context(tc.tile_pool(name="aotp", bufs=1))
            wo_pool = top.enter_context(tc.tile_pool(name="wo", bufs=1))
            mk_pool = top.enter_context(tc.tile_pool(name="mk", bufs=1))
            e_pool = top.enter_context(tc.tile_pool(name="ep", bufs=6))
            at_pool = top.enter_context(tc.tile_pool(name="at", bufs=2))
            an_pool = top.enter_context(tc.tile_pool(name="an", bufs=2))
            QT = [qkv_pool.tile([128, S], F32R, tag=f"qt{h}", name=f"qt{h}")
                  for h in range(H_LOC)]
            KT = qkv_pool.tile([128, S], F32R, tag="kt")
            VT = vt_pool.tile([128, S], BF16, tag="vtt")   # V.T (bf16)
            # V natural + ones column, per k-tile regions of width VSP
            Vn = vt_pool.tile([128, NKB * VSP], BF16, tag="vn")
            nc.vector.memset(Vn[:], 1.0)   # ones col at kb*VSP+128 survives

            # =================== Phase 1: QKV projections + RoPE ===================
            with ExitStack() as p1:
                w_pool = p1.enter_context(tc.tile_pool(name="wqkv", bufs=1))
                trig = p1.enter_context(tc.tile_pool(name="trig", bufs=1))
                # bufs=3: with 2, the next qb's first chunk DMA serializes
                # behind this qb's J=24..27 matmuls (slot WAR) and the PE
                # then eats the full transfer+receipt latency (~6us per
                # qb boundary)
                xc_pool = p1.enter_context(tc.tile_pool(name="xc", bufs=3))
                rope_t = p1.enter_context(tc.tile_pool(name="rope", bufs=1))
                ps1 = p1.enter_context(tc.tile_pool(name="ps1", bufs=1, space="PSUM"))

                # W.T in 8 grouped loads (4 J-tiles each, 1.5 MB) on sync.
                WG = []
                for g in range(8):
                    wg = w_pool.tile([128, 4 * 768], BF16, tag=f"wg{g}", name=f"wg{g}")
                    if g == 0:
                        # split the first group in two so the very first
                        # matmuls (J0/J1) start ~2us earlier
                        for hh in range(2):
                            s2 = wqkvT_d.ap()[hh * 256:(hh + 1) * 256, :]
                            nc.sync.dma_start(
                                wg[:, hh * 1536:(hh + 1) * 1536].rearrange(
                                    "p (j c) -> p j c", j=2),
                                s2.rearrange("(j p) c -> p j c", j=2))
                    else:
                        src = wqkvT_d.ap()[g * 512:(g + 1) * 512, :]
                        nc.sync.dma_start(
                            wg[:].rearrange("p (j c) -> p j c", j=4),
                            src.rearrange("(j p) c -> p j c", j=4))
                    WG.append(wg)

                def Wsl(J, lo, hi):
                    g, j = J // 4, J % 4
                    return WG[g][:, j * 768 + lo:j * 768 + hi]

                # trig tiles; sign of rotate-half folded into sinTs rows
                # [0:64). On gpsimd: the HWDGE queues must deliver W/x first
                # (the first rope is ~30us in, gpsimd delivers these by ~15us)
                cosT = trig.tile([128, S], F32, tag="cosT")
                nc.gpsimd.dma_start(cosT[:], cosT_d.ap())
                sinTs = trig.tile([128, S], F32, tag="sinTs")
                nc.gpsimd.dma_start(sinTs[:], sinT_d.ap())
                nc.vector.tensor_scalar(sinTs[0:64, :], sinTs[0:64, :], -1.0, None,
                                        AOT.mult)

                for qb in range(NQB):
                    sl = slice(qb * QB, (qb + 1) * QB)
                    # x chunks: 8 per qb, 4 J-tiles each ([128, 2048] = 1 MB),
                    # alternating HWDGE queues.
                    xcs = []
                    for cidx in range(8):
                        xc = xc_pool.tile([128, 2048], BF16, tag="xc", name="xc")
                        src = xT_d.ap()[cidx * 512:(cidx + 1) * 512, sl]
                        src = src.rearrange("(j p) c -> p j c", j=4)
                        # qb0: keep x off the sync queue (it is serially
                        # draining the 34us of W-group loads); later qbs
                        # alternate queues.
                        if qb == 0:
                            eng = nc.scalar
                        else:
                            eng = nc.scalar if (qb * 8 + cidx) % 2 else nc.sync
                        eng.dma_start(xc[:].rearrange("p (j c) -> p j c", j=4),
                                      src)
                        xcs.append(xc)

                    # psq0/psq1 double-buffered: the next qb's first matmuls
                    # start in fresh banks while this qb's evacuations drain
                    psQ = [ps1.tile([128, QB], F32, tag=f"psq{h}", name=f"psq{h}",
                                    bufs=(2 if h < 2 else 1))
                           for h in range(H_LOC)]
                    psK = ps1.tile([128, QB], F32, tag="psk")
                    psV = ps1.tile([128, QB], F32, tag="psv")
                    for J in range(32):
                        xt = xcs[J // 4][:, (J % 4) * 512:(J % 4 + 1) * 512]
                        st, sp = (J == 0), (J == 31)
                        for h in range(H_LOC):
                            nc.tensor.matmul(psQ[h][:],
                                             Wsl(J, h * 128, (h + 1) * 128),
                                             xt, start=st, stop=sp)
                        nc.tensor.matmul(psK[:], Wsl(J, 512, 640), xt,
                                         start=st, stop=sp)
                        nc.tensor.matmul(psV[:], Wsl(J, 640, 768), xt,
                                         start=st, stop=sp)

                    # Evacuate all PSUM accumulators first (frees banks for
                    # the next qb's matmuls) on the ACT engine, so the DVE
                    # rope math below never blocks the PE's next qb.
                    # Single-buffered banks (psq2/psq3/psK/psV) evacuate
                    # first: they gate the next qb's first matmuls.
                    raws = [None] * H_LOC
                    for h in (2, 3):
                        raw = rope_t.tile([128, QB], F32, tag=f"raw{h}",
                                          name=f"raw{h}", bufs=1)
                        # DVE: runs in parallel with the ACT evacs below, so
                        # the single-buffered banks free ~2x faster
                        nc.vector.tensor_copy(raw[:], psQ[h][:])
                        raws[h] = raw
                    rawk = rope_t.tile([128, QB], F32, tag="rawk", name="rawk",
                                       bufs=1)
                    nc.scalar.activation(rawk[:], psK[:], AFT.Copy)
                    nc.scalar.activation(VT[:, sl], psV[:], AFT.Copy)
                    for h in (0, 1):
                        raw = rope_t.tile([128, QB], F32, tag=f"raw{h}",
                                          name=f"raw{h}", bufs=2)
                        nc.scalar.activation(raw[:], psQ[h][:], AFT.Copy)
                        raws[h] = raw

                    def rope_finish(raw, dst):
                        rot = rope_t.tile([128, QB], F32, tag="rot", name="rot",
                                          bufs=2)
                        # gpsimd issue queue: idle in phase 1, and NOT behind
                        # the weight/x chunk DMAs on the HWDGE queues
                        nc.gpsimd.dma_start(rot[0:64, :], raw[64:128, :])
                        nc.gpsimd.dma_start(rot[64:128, :], raw[0:64, :])
                        t1 = rope_t.tile([128, QB], F32, tag="t1", name="t1",
                                         bufs=2)
                        nc.vector.tensor_tensor(t1[:], raw[:], cosT[:, sl], AOT.mult)
                        nc.vector.tensor_tensor(rot[:], rot[:], sinTs[:, sl],
                                                AOT.mult)
                        nc.vector.tensor_tensor(dst[:, sl], t1[:], rot[:], AOT.add)

                    for h in range(H_LOC):
                        rope_finish(raws[h], QT[h])
                    rope_finish(rawk, KT)

            # =================== Phase 2: attention + output projection ===========
            AOT_t = [aot_pool.tile([128, S], BF16, tag=f"aot{h}", name=f"aot{h}")
                     for h in range(H_LOC)]

            with ExitStack() as p2:
                mkg_pool = p2.enter_context(tc.tile_pool(name="mkg", bufs=4))
                stage_holder = []
                ps_s = p2.enter_context(tc.tile_pool(name="ps_s", bufs=2, space="PSUM"))
                ps_av = p2.enter_context(tc.tile_pool(name="ps_av", bufs=2, space="PSUM"))
                ps_o = p2.enter_context(tc.tile_pool(name="ps_o", bufs=2, space="PSUM"))
                ps_t = p2.enter_context(tc.tile_pool(name="ps_t", bufs=1, space="PSUM"))

                # causal: 0/1 bf16 diagonal-tile masks generated on-device.
                # mask01[i][p, c] = 1.0 iff c - p >= i*128
                mtiles = []
                if causal:
                    iot = mk_pool.tile([128, QB], I32, tag="iot")
                    nc.gpsimd.iota(iot[:], pattern=[[1, QB]], base=0,
                                   channel_multiplier=-1)
                    for i in range(4):
                        mt = mk_pool.tile([128, QB], BF16, tag=f"mk{i}",
                                          name=f"mk{i}")
                        nc.vector.tensor_scalar(mt[:], iot[:], i * 128, None,
                                                AOT.is_ge)
                        mtiles.append(mt)

                # Wo (bf16, host-dequantized) resident: 4 head-tiles
                # [128, 4096], split across 3 queues so the first proj
                # (~15 us into phase 2) never waits.
                WOt = []
                for J in range(H_LOC):
                    wot = wo_pool.tile([128, DIM], BF16, tag=f"wo{J}", name=f"wo{J}")
                    eng = (nc.sync, nc.scalar, nc.gpsimd, nc.sync)[J]
                    eng.dma_start(wot[:], woT_d.ap()[J * 128:(J + 1) * 128, :])
                    WOt.append(wot)

                def proj_chunk(sb, feeder=None):
                    """Output projection + staged bf16 write for one 128-row
                    seq block. `feeder(1)` is called between ob groups so the
                    score->exp pipeline keeps filling while the PE runs proj
                    (otherwise ACT idles during proj and the PE starves on
                    exp afterwards)."""
                    ssl = slice(sb * 128, (sb + 1) * 128)
                    if not stage_holder:
                        stage_holder.append(
                            p2.enter_context(tc.tile_pool(name="stage", bufs=2)))
                    stg = stage_holder[0].tile([128, DIM], BF16, tag="stg",
                                               name="stg")
                    for ob in range(DIM // 512):
                        osl = slice(ob * 512, (ob + 1) * 512)
                        psO = ps_o.tile([128, 512], F32, tag="pso", name="pso")
                        for J in range(H_LOC):
                            nc.tensor.matmul(psO[:], AOT_t[J][:, ssl],
                                             WOt[J][:, osl],
                                             start=(J == 0), stop=(J == 3))
                        nc.vector.tensor_copy(stg[:, osl], psO[:])
                        if feeder is not None and ob % 2 == 1:
                            feeder(1)
                    # SWDGE: gpsimd is idle in phase 2 and this keeps the
                    # two HWDGE queues free. The last 4 chunks go HWDGE
                    # (idle by then; ~1.4us less completion latency each on
                    # the kernel tail).
                    if sb >= 4 * (NQB - 1):
                        eng = nc.scalar if sb % 2 else nc.sync
                    else:
                        eng = nc.gpsimd
                    eng.dma_start(out_d.ap()[ssl, :], stg[:])

                def exp_tile(qb, h, kb):
                    sl = slice(qb * QB, (qb + 1) * QB)
                    psS = ps_s.tile([128, QB], F32, tag="pss", name="pss")
                    nc.tensor.matmul(psS[:],
                                     KT[:, kb * 128:(kb + 1) * 128],
                                     QT[h][:, sl], start=True, stop=True)
                    E = e_pool.tile([128, QB], BF16, tag="e", name="e")
                    nc.scalar.activation(E[:], psS[:], AFT.Exp,
                                         scale=float(SCALE))
                    if causal and kb >= 4 * qb:
                        nc.vector.tensor_tensor(E[:], E[:], mtiles[kb - 4 * qb][:],
                                                AOT.mult)
                    elif not causal:
                        mt = mkg_pool.tile([128, QB], F32, tag=f"mkg{kb % 4}",
                                           name=f"mkg{kb}")
                        nc.sync.dma_start(
                            mt[:], mask_d.ap()[kb * 128:(kb + 1) * 128, sl])
                        tmp = at_pool.tile([128, QB], F32, tag="sm", name="sm")
                        nc.vector.scalar_tensor_tensor(
                            tmp[:], psS[:], float(SCALE), mt[:],
                            AOT.mult, AOT.add)
                        nc.scalar.activation(E[:], tmp[:], AFT.Exp)
                    return E

                # Flat schedule over (qb, h, kb) with a global score/exp
                # pipeline LOOKAHEAD tiles deep -- the E queue stays primed
                # across head and q-block boundaries, so the PE's AV matmuls
                # never wait on the psS -> exp chain.
                flat = []
                for qb in range(NQB):
                    nkb = 4 * (qb + 1) if causal else NKB
                    for h in range(H_LOC):
                        for kb in range(nkb):
                            flat.append((qb, h, kb, nkb))
                LOOKAHEAD = 3
                EMAX = 5
                Equeue = []
                pidx = 0

                def feed(n):
                    # produce up to n lookahead score->exp tiles, bounded by
                    # the E pool depth
                    nonlocal pidx
                    emitted = 0
                    while (pidx < len(flat) and emitted < n
                           and len(Equeue) < EMAX):
                        pq, ph, pk, _ = flat[pidx]
                        Equeue.append(exp_tile(pq, ph, pk))
                        pidx += 1
                        emitted += 1

                def vt_transposes(qb):
                    # V.T -> V natural (bf16) on the PE (no DMA-xbar: its
                    # SBUF->SBUF-DMA hazard window opens under 8-core HBM
                    # contention), prefetched one q-block ahead.
                    for tkb in range(4 * qb, 4 * qb + 4):
                        pt = ps_t.tile([128, 128], BF16, tag="pt", name="pvt")
                        nc.tensor.transpose(
                            pt[:], VT[:, tkb * 128:(tkb + 1) * 128], identb[:])
                        nc.scalar.activation(
                            Vn[:, tkb * VSP:tkb * VSP + 128], pt[:], AFT.Copy)

                vt_transposes(0)
                feed(LOOKAHEAD + 1)
                psAV = [None, None]
                for idx, (qb, h, kb, nkb) in enumerate(flat):
                    if h == 0 and kb == 0 and qb + 1 < NQB:
                        vt_transposes(qb + 1)
                    if kb == 0:
                        # two banks, each holding two q-sub regions
                        # [q, hd | D] of width ASP
                        psAV[0] = ps_av.tile([128, 2 * ASP], F32, tag="av0",
                                             name="av0")
                        psAV[1] = ps_av.tile([128, 2 * ASP], F32, tag="av1",
                                             name="av1", bufs=1)
                    st, sp = (kb == 0), (kb == nkb - 1)
                    Ecur = Equeue.pop(0)
                    vsl = Vn[:, kb * VSP:kb * VSP + 129]
                    for qs in range(4):
                        # one accumulation group per PSUM bank: start marks
                        # the whole bank pending-zero, so only the first MM
                        # into the bank starts and only the last stops
                        nc.tensor.matmul(
                            psAV[qs // 2][:, (qs % 2) * ASP:(qs % 2) * ASP + 129],
                            Ecur[:, qs * 128:(qs + 1) * 128],
                            vsl,
                            start=(st and qs % 2 == 0),
                            stop=(sp and qs % 2 == 1))
                    if sp:
                        # normalize (reciprocal of the folded D column as a
                        # per-partition ACT scale) + DMA-xbar transpose back
                        # to [hd, q] per q-sub
                        rec = at_pool.tile([128, 4], F32, tag="rec", name="rec")
                        for qs in range(4):
                            reg = psAV[qs // 2][:, (qs % 2) * ASP:
                                                (qs % 2) * ASP + 129]
                            nc.vector.reciprocal(rec[:, qs:qs + 1],
                                                 reg[:, 128:129])
                            An = an_pool.tile([128, 128], BF16, tag="an",
                                              name="an")
                            # DVE: ACT is saturated by the exp stream
                            nc.vector.tensor_scalar(An[:], reg[:, 0:128],
                                                    rec[:, qs:qs + 1], None,
                                                    AOT.mult)
                            pt = ps_t.tile([128, 128], BF16, tag="pt",
                                           name="pat")
                            nc.tensor.transpose(pt[:], An[:], identb[:])
                            nc.scalar.activation(
                                AOT_t[h][:, qb * QB + qs * 128:
                                         qb * QB + (qs + 1) * 128],
                                pt[:], AFT.Copy)
                        # interleave the previous q-block's output projection:
                        # fills the normalize/transpose latency with PE work.
                        if qb > 0:
                            proj_chunk(4 * (qb - 1) + h, feeder=feed)
                    # maintain the lookahead AFTER consumption, so the next
                    # tiles' exps queue behind this tile's ACT work
                    while (pidx < len(flat) and pidx <= idx + 1 + LOOKAHEAD
                           and len(Equeue) < EMAX):
                        pq, ph, pk, _ = flat[pidx]
                        Equeue.append(exp_tile(pq, ph, pk))
                        pidx += 1
                for i in range(4):
                    proj_chunk(4 * (NQB - 1) + i)

                if dump:
                    nc.gpsimd.dma_start(dbg["qt0"].ap(), QT[0][:].bitcast(F32))
                    nc.gpsimd.dma_start(dbg["kt"].ap(), KT[:].bitcast(F32))
                    nc.gpsimd.dma_start(dbg["vt"].ap(), VT[:])
                    nc.gpsimd.dma_start(dbg["vn"].ap(), Vn[:])
                    nc.gpsimd.dma_start(dbg["aot0"].ap(), AOT_t[0][:])

    nc.compile()
    return nc


_BUILD_CACHE = {}


def _get_kernel(causal: bool):
    if causal not in _BUILD_CACHE:
        _BUILD_CACHE[causal] = _build_kernel(causal)
    return _BUILD_CACHE[causal]


def _dequant_np(packed, scales, out_f, in_f):
    """Exact numpy port of the reference dequantize_q40."""
    w = np.asarray(packed)
    if w.dtype != np.int8:
        w = w.astype(np.int8)
    s = np.asarray(scales, dtype=np.float32).reshape(-1, 1)
    msb = w >> 4                                   # arithmetic, sign-extends
    lsb = (w << 4) >> 4                            # int8 wraps, then sign-extends
    grp = np.concatenate([msb, lsb], axis=1).reshape(-1, GROUP).astype(np.float32)
    return (grp * s).reshape(out_f, in_f)


def _canonical_causal_mask():
    causal = np.triu(np.ones((S, S), dtype=bool), k=1)
    return np.where(causal, np.float32(NEG), np.float32(0.0)).astype(np.float32)


def build_in_maps(inputs, causal):
    x = np.asarray(inputs["x"], dtype=np.float32)
    cos = np.asarray(inputs["cos"], dtype=np.float32)
    sin = np.asarray(inputs["sin"], dtype=np.float32)

    xT = np.ascontiguousarray(x.reshape(S, DIM).T).astype(ml_dtypes.bfloat16)
    cosT = np.ascontiguousarray(np.concatenate([cos.T, cos.T], axis=0))  # [128, S]
    sinT = np.ascontiguousarray(np.concatenate([sin.T, sin.T], axis=0))

    Wq = _dequant_np(inputs["wq"], inputs["sq"], N_HEADS * HEAD_DIM, DIM)
    Wk = _dequant_np(inputs["wk"], inputs["sk"], N_KV * HEAD_DIM, DIM)
    Wv = _dequant_np(inputs["wv"], inputs["sv"], N_KV * HEAD_DIM, DIM)
    Wo = _dequant_np(inputs["wo"], inputs["so"], DIM, N_HEADS * HEAD_DIM)

    in_maps = []
    for c in range(NCORES):
        q0 = c * QDIM_LOC
        k0 = c * HEAD_DIM
        wqkvT = np.empty((DIM, 768), dtype=ml_dtypes.bfloat16)
        wqkvT[:, 0:512] = Wq[q0:q0 + QDIM_LOC].T.astype(ml_dtypes.bfloat16)
        wqkvT[:, 512:640] = Wk[k0:k0 + HEAD_DIM].T.astype(ml_dtypes.bfloat16)
        wqkvT[:, 640:768] = Wv[k0:k0 + HEAD_DIM].T.astype(ml_dtypes.bfloat16)
        woT = np.ascontiguousarray(
            Wo[:, q0:q0 + QDIM_LOC].T).astype(ml_dtypes.bfloat16)  # [512, 4096]
        m = dict(xT=xT, wqkvT=wqkvT, woT=woT, cosT=cosT, sinT=sinT)
        if not causal:
            mask = np.asarray(inputs["mask"], dtype=np.float32)
            m["maskT"] = np.ascontiguousarray(mask.T)
        in_maps.append(m)
    return in_maps


def kernel(**inputs):
    mask = np.asarray(inputs["mask"], dtype=np.float32)
    causal = bool(np.array_equal(mask, _canonical_causal_mask()))
    nc = _get_kernel(causal)
    in_maps = build_in_maps(inputs, causal)
    res = bass_utils.run_bass_kernel_spmd(nc, in_maps, core_ids=list(range(NCORES)))
    acc = np.zeros((S, DIM), dtype=np.float32)
    for r in res.results:
        acc += np.asarray(r["out_partial"]).astype(np.float32)
    return acc.reshape(1, S, DIM)


if __name__ == "__main__":
    print("building causal kernel...")
    _get_kernel(True)
    print("built")


# revision 52
# speedup vs baseline: 1.0117x; 1.0017x over previous
"""Trainium2 Bass kernel for quantized (Q4_0) multi-head attention prefill.

Problem: nn_Attention_32023276159509
  B=1, S=2048, DIM=4096, 32 q-heads / 8 kv-heads (GQA x4), head_dim=128,
  Q4_0-packed int4 weights with per-64-group fp32 scales, RoPE (rotate-half),
  causal mask, softmax, output projection.

Sharding: tensor-parallel over heads across 8 NeuronCores. Core c owns
q-heads [4c, 4c+4), kv-head c, and wo input-columns [512c, 512(c+1)).
Each core computes a full [S, DIM] partial output; partials are summed on
the host (the all-reduce of the reference sharding recipe).

v3 design (vs v2 baseline, 607 us):
  - Phase-1 x stream: 1 MB 4-J-chunk DMAs (8 KB/partition descriptors per
    chunk via a rearranged AP) instead of 256 KB tile pairs. The per-DMA
    fixed cost amortizes: ~340 GB/s burst vs the ~133 GB/s that made
    phase 1 DMA-bound.
  - Weights in 8 grouped 1.5 MB loads on the sync queue.
  - D (softmax denominator) folded into the AV matmul: E is the stationary
    operand, V gets a ones column appended ([128,129] moving operand), so
    the unnormalized attention output lands as [q, hd | D] in PSUM. This
    kills 160 separate row-sum matmuls AND the [1,512] single-partition
    reciprocal (3.3 us each!) + partition_broadcast chain: the reciprocal
    is now a [128,1] per-partition op and the normalize is folded into the
    ACT-engine PSUM evacuation as a per-partition scale.
  - The [q, hd] result is transposed back to [hd, q] for the output
    projection with PE transposes that reuse the same PSUM bank in place.
  - Causal mask generated on-device with iota (0/1 bf16, multiplied into E
    post-exp) -- no mask DMA, no fp32 mask-add on the DVE.
  - wo prefetched at phase-2 open split across sync/scalar/gpsimd queues.
  - Output partials written bf16 as 16 x 1 MB HWDGE DMAs (was 128 x 256 KB
    SWDGE writes whose ~2.6 us serialized fixed costs bounded phase 2).
"""
import sys
import numpy as np

sys.path.insert(0, "/opt/trn_rl_repo")

import concourse.bass as bass  # noqa: E402
import concourse.tile as tile  # noqa: E402
from concourse import bacc, mybir, bass_utils  # noqa: E402
from contextlib import ExitStack  # noqa: E402
import ml_dtypes  # noqa: E402

F32 = mybir.dt.float32
F32R = mybir.dt.float32r
BF16 = mybir.dt.bfloat16
I32 = mybir.dt.int32
AOT = mybir.AluOpType
AFT = mybir.ActivationFunctionType

GROUP = 64
DIM = 4096
N_HEADS = 32
N_KV = 8
HEAD_DIM = 128
S = 2048
NCORES = 8
H_LOC = N_HEADS // NCORES          # 4 local q heads
QDIM_LOC = H_LOC * HEAD_DIM        # 512
SCALE = 1.0 / np.sqrt(np.float32(HEAD_DIM))
NEG = -1e9

QB = 512                            # q-block (seq columns per attention tile)
NQB = S // QB                       # 4
NKB = S // 128                      # 16 k-tiles of 128
VSP = 132                           # Vn_aug column stride per k-tile (129 used)
ASP = 132                           # psAV region stride (129 cols used)


def _build_kernel(causal: bool, dump: bool = False):
    """Build + compile the per-core Bass module. Same program on all cores."""
    nc = bacc.Bacc("TRN2", target_bir_lowering=False, debug=False)
    dbg = {}
    if dump:
        dbg["qt0"] = nc.dram_tensor("d_qt0", [128, S], F32, kind="ExternalOutput")
        dbg["kt"] = nc.dram_tensor("d_kt", [128, S], F32, kind="ExternalOutput")
        dbg["vt"] = nc.dram_tensor("d_vt", [128, S], BF16, kind="ExternalOutput")
        dbg["vn"] = nc.dram_tensor("d_vn", [128, NKB * VSP], BF16,
                                   kind="ExternalOutput")
        dbg["aot0"] = nc.dram_tensor("d_aot0", [128, S], BF16,
                                     kind="ExternalOutput")

    # ---- DRAM tensors (per-core inputs) ----
    # x and wqkv in bf16: the f32r LDWEIGHTS (224 ns) otherwise gates every
    # 216 ns phase-1 matmul; bf16 weights get FWL (~115-180 ns, hidden).
    xT_d = nc.dram_tensor("xT", [DIM, S], BF16, kind="ExternalInput")
    wqkvT_d = nc.dram_tensor("wqkvT", [DIM, 768], BF16, kind="ExternalInput")
    woT_d = nc.dram_tensor("woT", [QDIM_LOC, DIM], BF16, kind="ExternalInput")
    cosT_d = nc.dram_tensor("cosT", [128, S], F32, kind="ExternalInput")
    sinT_d = nc.dram_tensor("sinT", [128, S], F32, kind="ExternalInput")
    if not causal:
        mask_d = nc.dram_tensor("maskT", [S, S], F32, kind="ExternalInput")
    out_d = nc.dram_tensor("out_partial", [S, DIM], BF16, kind="ExternalOutput")

    with tile.TileContext(nc) as tc:
        with ExitStack() as top:
            # ---- persistent small constants ----
            cpool = top.enter_context(tc.tile_pool(name="const", bufs=1))
            iden_i = cpool.tile([128, 128], I32, tag="iden_i")
            nc.gpsimd.iota(iden_i[:], pattern=[[1, 128]], base=0,
                           channel_multiplier=-1)
            identb = cpool.tile([128, 128], BF16, tag="identb")
            nc.vector.tensor_scalar(identb[:], iden_i[:], 0, None, AOT.is_equal)

            # ---- persistent activations ----
            # All phase-2 SBUF pools are allocated TOP-LEVEL (before the
            # phase-1 pools): in stack order their space never overlaps the
            # phase-1 pools, so phase-2's first tiles don't wait for the
            # phase-1 release chain (which ends with qb3's rope + rot DMAs,
            # ~15us after the last phase-1 matmul). Only `stage` is entered
            # lazily at the first projection, by which time phase-1 space is
            # long free.
            qkv_pool = top.enter_context(tc.tile_pool(name="qkv", bufs=1))
            vt_pool = top.enter_context(tc.tile_pool(name="vt", bufs=1))
            aot_pool = top.enter_context(tc.tile_pool(name="aotp", bufs=1))
            wo_pool = top.enter_context(tc.tile_pool(name="wo", bufs=1))
            mk_pool = top.enter_context(tc.tile_pool(name="mk", bufs=1))
            e_pool = top.enter_context(tc.tile_pool(name="ep", bufs=6))
            at_pool = top.enter_context(tc.tile_pool(name="at", bufs=2))
            an_pool = top.enter_context(tc.tile_pool(name="an", bufs=2))
            QT = [qkv_pool.tile([128, S], F32R, tag=f"qt{h}", name=f"qt{h}")
                  for h in range(H_LOC)]
            KT = qkv_pool.tile([128, S], F32R, tag="kt")
            VT = vt_pool.tile([128, S], BF16, tag="vtt")   # V.T (bf16)
            # V natural + ones column, per k-tile regions of width VSP
            Vn = vt_pool.tile([128, NKB * VSP], BF16, tag="vn")
            nc.vector.memset(Vn[:], 1.0)   # ones col at kb*VSP+128 survives

            # =================== Phase 1: QKV projections + RoPE ===================
            with ExitStack() as p1:
                w_pool = p1.enter_context(tc.tile_pool(name="wqkv", bufs=1))
                trig = p1.enter_context(tc.tile_pool(name="trig", bufs=1))
                # bufs=3: with 2, the next qb's first chunk DMA serializes
                # behind this qb's J=24..27 matmuls (slot WAR) and the PE
                # then eats the full transfer+receipt latency (~6us per
                # qb boundary)
                xc_pool = p1.enter_context(tc.tile_pool(name="xc", bufs=3))
                rope_t = p1.enter_context(tc.tile_pool(name="rope", bufs=1))
                ps1 = p1.enter_context(tc.tile_pool(name="ps1", bufs=1, space="PSUM"))

                # W.T in 8 grouped loads (4 J-tiles each, 1.5 MB) on sync.
                WG = []
                for g in range(8):
                    wg = w_pool.tile([128, 4 * 768], BF16, tag=f"wg{g}", name=f"wg{g}")
                    if g == 0:
                        # split the first group in two so the very first
                        # matmuls (J0/J1) start ~2us earlier
                        for hh in range(2):
                            s2 = wqkvT_d.ap()[hh * 256:(hh + 1) * 256, :]
                            nc.sync.dma_start(
                                wg[:, hh * 1536:(hh + 1) * 1536].rearrange(
                                    "p (j c) -> p j c", j=2),
                                s2.rearrange("(j p) c -> p j c", j=2))
                    else:
                        src = wqkvT_d.ap()[g * 512:(g + 1) * 512, :]
                        nc.sync.dma_start(
                            wg[:].rearrange("p (j c) -> p j c", j=4),
                            src.rearrange("(j p) c -> p j c", j=4))
                    WG.append(wg)

                def Wsl(J, lo, hi):
                    g, j = J // 4, J % 4
                    return WG[g][:, j * 768 + lo:j * 768 + hi]

                # trig tiles; sign of rotate-half folded into sinTs rows
                # [0:64). On gpsimd: the HWDGE queues must deliver W/x first
                # (the first rope is ~30us in, gpsimd delivers these by ~15us)
                cosT = trig.tile([128, S], F32, tag="cosT")
                nc.gpsimd.dma_start(cosT[:], cosT_d.ap())
                sinTs = trig.tile([128, S], F32, tag="sinTs")
                nc.gpsimd.dma_start(sinTs[:], sinT_d.ap())
                nc.vector.tensor_scalar(sinTs[0:64, :], sinTs[0:64, :], -1.0, None,
                                        AOT.mult)

                for qb in range(NQB):
                    sl = slice(qb * QB, (qb + 1) * QB)
                    # x chunks: 8 per qb, 4 J-tiles each ([128, 2048] = 1 MB),
                    # alternating HWDGE queues.
                    xcs = []
                    for cidx in range(8):
                        xc = xc_pool.tile([128, 2048], BF16, tag="xc", name="xc")
                        src = xT_d.ap()[cidx * 512:(cidx + 1) * 512, sl]
                        src = src.rearrange("(j p) c -> p j c", j=4)
                        # qb0: keep x off the sync queue (it is serially
                        # draining the 34us of W-group loads); later qbs
                        # alternate queues.
                        if qb == 0:
                            eng = nc.scalar
                        else:
                            eng = nc.scalar if (qb * 8 + cidx) % 2 else nc.sync
                        eng.dma_start(xc[:].rearrange("p (j c) -> p j c", j=4),
                                      src)
                        xcs.append(xc)

                    # psq0/psq1 double-buffered: the next qb's first matmuls
                    # start in fresh banks while this qb's evacuations drain
                    psQ = [ps1.tile([128, QB], F32, tag=f"psq{h}", name=f"psq{h}",
                                    bufs=(2 if h < 2 else 1))
                           for h in range(H_LOC)]
                    psK = ps1.tile([128, QB], F32, tag="psk")
                    psV = ps1.tile([128, QB], F32, tag="psv")
                    for J in range(32):
                        xt = xcs[J // 4][:, (J % 4) * 512:(J % 4 + 1) * 512]
                        st, sp = (J == 0), (J == 31)
                        for h in range(H_LOC):
                            nc.tensor.matmul(psQ[h][:],
                                             Wsl(J, h * 128, (h + 1) * 128),
                                             xt, start=st, stop=sp)
                        nc.tensor.matmul(psK[:], Wsl(J, 512, 640), xt,
                                         start=st, stop=sp)
                        nc.tensor.matmul(psV[:], Wsl(J, 640, 768), xt,
                                         start=st, stop=sp)

                    # Evacuate all PSUM accumulators first (frees banks for
                    # the next qb's matmuls) on the ACT engine, so the DVE
                    # rope math below never blocks the PE's next qb.
                    # Single-buffered banks (psq2/psq3/psK/psV) evacuate
                    # first: they gate the next qb's first matmuls.
                    raws = [None] * H_LOC
                    for h in (2, 3):
                        raw = rope_t.tile([128, QB], F32, tag=f"raw{h}",
                                          name=f"raw{h}", bufs=1)
                        # DVE: runs in parallel with the ACT evacs below, so
                        # the single-buffered banks free ~2x faster
                        nc.vector.tensor_copy(raw[:], psQ[h][:])
                        raws[h] = raw
                    rawk = rope_t.tile([128, QB], F32, tag="rawk", name="rawk",
                                       bufs=1)
                    nc.scalar.activation(rawk[:], psK[:], AFT.Copy)
                    nc.scalar.activation(VT[:, sl], psV[:], AFT.Copy)
                    for h in (0, 1):
                        raw = rope_t.tile([128, QB], F32, tag=f"raw{h}",
                                          name=f"raw{h}", bufs=2)
                        nc.scalar.activation(raw[:], psQ[h][:], AFT.Copy)
                        raws[h] = raw

                    def rope_finish(raw, dst):
                        rot = rope_t.tile([128, QB], F32, tag="rot", name="rot",
                                          bufs=2)
                        # gpsimd issue queue: idle in phase 1, and NOT behind
                        # the weight/x chunk DMAs on the HWDGE queues
                        nc.gpsimd.dma_start(rot[0:64, :], raw[64:128, :])
                        nc.gpsimd.dma_start(rot[64:128, :], raw[0:64, :])
                        t1 = rope_t.tile([128, QB], F32, tag="t1", name="t1",
                                         bufs=2)
                        nc.vector.tensor_tensor(t1[:], raw[:], cosT[:, sl], AOT.mult)
                        nc.vector.tensor_tensor(rot[:], rot[:], sinTs[:, sl],
                                                AOT.mult)
                        nc.vector.tensor_tensor(dst[:, sl], t1[:], rot[:], AOT.add)

                    for h in range(H_LOC):
                        rope_finish(raws[h], QT[h])
                    rope_finish(rawk, KT)

            # =================== Phase 2: attention + output projection ===========
            AOT_t = [aot_pool.tile([128, S], BF16, tag=f"aot{h}", name=f"aot{h}")
                     for h in range(H_LOC)]

            with ExitStack() as p2:
                mkg_pool = p2.enter_context(tc.tile_pool(name="mkg", bufs=4))
                stage_holder = []
                ps_s = p2.enter_context(tc.tile_pool(name="ps_s", bufs=2, space="PSUM"))
                ps_av = p2.enter_context(tc.tile_pool(name="ps_av", bufs=2, space="PSUM"))
                ps_o = p2.enter_context(tc.tile_pool(name="ps_o", bufs=2, space="PSUM"))
                ps_t = p2.enter_context(tc.tile_pool(name="ps_t", bufs=1, space="PSUM"))

                # causal: 0/1 bf16 diagonal-tile masks generated on-device.
                # mask01[i][p, c] = 1.0 iff c - p >= i*128
                mtiles = []
                if causal:
                    iot = mk_pool.tile([128, QB], I32, tag="iot")
                    nc.gpsimd.iota(iot[:], pattern=[[1, QB]], base=0,
                                   channel_multiplier=-1)
                    for i in range(4):
                        mt = mk_pool.tile([128, QB], BF16, tag=f"mk{i}",
                                          name=f"mk{i}")
                        nc.vector.tensor_scalar(mt[:], iot[:], i * 128, None,
                                                AOT.is_ge)
                        mtiles.append(mt)

                # Wo (bf16, host-dequantized) resident: 4 head-tiles
                # [128, 4096], split across 3 queues so the first proj
                # (~15 us into phase 2) never waits.
                WOt = []
                for J in range(H_LOC):
                    wot = wo_pool.tile([128, DIM], BF16, tag=f"wo{J}", name=f"wo{J}")
                    eng = (nc.sync, nc.scalar, nc.gpsimd, nc.sync)[J]
                    eng.dma_start(wot[:], woT_d.ap()[J * 128:(J + 1) * 128, :])
                    WOt.append(wot)

                def proj_chunk(sb, feeder=None):
                    """Output projection + staged bf16 write for one 128-row
                    seq block. `feeder(1)` is called between ob groups so the
                    score->exp pipeline keeps filling while the PE runs proj
                    (otherwise ACT idles during proj and the PE starves on
                    exp afterwards)."""
                    ssl = slice(sb * 128, (sb + 1) * 128)
                    if not stage_holder:
                        stage_holder.append(
                            p2.enter_context(tc.tile_pool(name="stage", bufs=2)))
                    stg = stage_holder[0].tile([128, DIM], BF16, tag="stg",
                                               name="stg")
                    for ob in range(DIM // 512):
                        osl = slice(ob * 512, (ob + 1) * 512)
                        psO = ps_o.tile([128, 512], F32, tag="pso", name="pso")
                        for J in range(H_LOC):
                            nc.tensor.matmul(psO[:], AOT_t[J][:, ssl],
                                             WOt[J][:, osl],
                                             start=(J == 0), stop=(J == 3))
                        nc.vector.tensor_copy(stg[:, osl], psO[:])
                        if feeder is not None and ob % 2 == 1:
                            feeder(1)
                    # SWDGE: gpsimd is idle in phase 2 and this keeps the
                    # two HWDGE queues free. The last 4 chunks go HWDGE
                    # (idle by then; ~1.4us less completion latency each on
                    # the kernel tail).
                    if sb == 4 * NQB - 1:
                        # very last chunk: write in halves so the first half
                        # (deps: obs 0-3) fires while obs 4-7 still project,
                        # halving the exposed transfer on the kernel tail
                        half = DIM // 2
                        nc.scalar.dma_start(out_d.ap()[ssl, 0:half],
                                            stg[:, 0:half])
                        nc.sync.dma_start(out_d.ap()[ssl, half:],
                                          stg[:, half:])
                    elif sb >= 4 * (NQB - 1):
                        eng = nc.scalar if sb % 2 else nc.sync
                        eng.dma_start(out_d.ap()[ssl, :], stg[:])
                    else:
                        nc.gpsimd.dma_start(out_d.ap()[ssl, :], stg[:])

                def exp_tile(qb, h, kb):
                    sl = slice(qb * QB, (qb + 1) * QB)
                    psS = ps_s.tile([128, QB], F32, tag="pss", name="pss")
                    nc.tensor.matmul(psS[:],
                                     KT[:, kb * 128:(kb + 1) * 128],
                                     QT[h][:, sl], start=True, stop=True)
                    E = e_pool.tile([128, QB], BF16, tag="e", name="e")
                    nc.scalar.activation(E[:], psS[:], AFT.Exp,
                                         scale=float(SCALE))
                    if causal and kb >= 4 * qb:
                        nc.vector.tensor_tensor(E[:], E[:], mtiles[kb - 4 * qb][:],
                                                AOT.mult)
                    elif not causal:
                        mt = mkg_pool.tile([128, QB], F32, tag=f"mkg{kb % 4}",
                                           name=f"mkg{kb}")
                        nc.sync.dma_start(
                            mt[:], mask_d.ap()[kb * 128:(kb + 1) * 128, sl])
                        tmp = at_pool.tile([128, QB], F32, tag="sm", name="sm")
                        nc.vector.scalar_tensor_tensor(
                            tmp[:], psS[:], float(SCALE), mt[:],
                            AOT.mult, AOT.add)
                        nc.scalar.activation(E[:], tmp[:], AFT.Exp)
                    return E

                # Flat schedule over (qb, h, kb) with a global score/exp
                # pipeline LOOKAHEAD tiles deep -- the E queue stays primed
                # across head and q-block boundaries, so the PE's AV matmuls
                # never wait on the psS -> exp chain.
                flat = []
                for qb in range(NQB):
                    nkb = 4 * (qb + 1) if causal else NKB
                    for h in range(H_LOC):
                        for kb in range(nkb):
                            flat.append((qb, h, kb, nkb))
                LOOKAHEAD = 3
                EMAX = 5
                Equeue = []
                pidx = 0

                def feed(n):
                    # produce up to n lookahead score->exp tiles, bounded by
                    # the E pool depth
                    nonlocal pidx
                    emitted = 0
                    while (pidx < len(flat) and emitted < n
                           and len(Equeue) < EMAX):
                        pq, ph, pk, _ = flat[pidx]
                        Equeue.append(exp_tile(pq, ph, pk))
                        pidx += 1
                        emitted += 1

                def vt_transposes(qb):
                    # V.T -> V natural (bf16) on the PE (no DMA-xbar: its
                    # SBUF->SBUF-DMA hazard window opens under 8-core HBM
                    # contention), prefetched one q-block ahead.
                    for tkb in range(4 * qb, 4 * qb + 4):
                        pt = ps_t.tile([128, 128], BF16, tag="pt", name="pvt")
                        nc.tensor.transpose(
                            pt[:], VT[:, tkb * 128:(tkb + 1) * 128], identb[:])
                        nc.scalar.activation(
                            Vn[:, tkb * VSP:tkb * VSP + 128], pt[:], AFT.Copy)

                vt_transposes(0)
                feed(LOOKAHEAD + 1)
                psAV = [None, None]
                for idx, (qb, h, kb, nkb) in enumerate(flat):
                    if h == 0 and kb == 0 and qb + 1 < NQB:
                        vt_transposes(qb + 1)
                    if kb == 0:
                        # two banks, each holding two q-sub regions
                        # [q, hd | D] of width ASP
                        psAV[0] = ps_av.tile([128, 2 * ASP], F32, tag="av0",
                                             name="av0")
                        psAV[1] = ps_av.tile([128, 2 * ASP], F32, tag="av1",
                                             name="av1", bufs=1)
                    st, sp = (kb == 0), (kb == nkb - 1)
                    Ecur = Equeue.pop(0)
                    vsl = Vn[:, kb * VSP:kb * VSP + 129]
                    for qs in range(4):
                        # one accumulation group per PSUM bank: start marks
                        # the whole bank pending-zero, so only the first MM
                        # into the bank starts and only the last stops
                        nc.tensor.matmul(
                            psAV[qs // 2][:, (qs % 2) * ASP:(qs % 2) * ASP + 129],
                            Ecur[:, qs * 128:(qs + 1) * 128],
                            vsl,
                            start=(st and qs % 2 == 0),
                            stop=(sp and qs % 2 == 1))
                    if sp:
                        # normalize (reciprocal of the folded D column as a
                        # per-partition ACT scale) + DMA-xbar transpose back
                        # to [hd, q] per q-sub
                        rec = at_pool.tile([128, 4], F32, tag="rec", name="rec")
                        for qs in range(4):
                            reg = psAV[qs // 2][:, (qs % 2) * ASP:
                                                (qs % 2) * ASP + 129]
                            nc.vector.reciprocal(rec[:, qs:qs + 1],
                                                 reg[:, 128:129])
                            An = an_pool.tile([128, 128], BF16, tag="an",
                                              name="an")
                            # DVE: ACT is saturated by the exp stream
                            nc.vector.tensor_scalar(An[:], reg[:, 0:128],
                                                    rec[:, qs:qs + 1], None,
                                                    AOT.mult)
                            pt = ps_t.tile([128, 128], BF16, tag="pt",
                                           name="pat")
                            nc.tensor.transpose(pt[:], An[:], identb[:])
                            nc.scalar.activation(
                                AOT_t[h][:, qb * QB + qs * 128:
                                         qb * QB + (qs + 1) * 128],
                                pt[:], AFT.Copy)
                        # interleave the previous q-block's output projection:
                        # fills the normalize/transpose latency with PE work.
                        if qb > 0:
                            proj_chunk(4 * (qb - 1) + h, feeder=feed)
                    # maintain the lookahead AFTER consumption, so the next
                    # tiles' exps queue behind this tile's ACT work
                    while (pidx < len(flat) and pidx <= idx + 1 + LOOKAHEAD
                           and len(Equeue) < EMAX):
                        pq, ph, pk, _ = flat[pidx]
                        Equeue.append(exp_tile(pq, ph, pk))
                        pidx += 1
                for i in range(4):
                    proj_chunk(4 * (NQB - 1) + i)

                if dump:
                    nc.gpsimd.dma_start(dbg["qt0"].ap(), QT[0][:].bitcast(F32))
                    nc.gpsimd.dma_start(dbg["kt"].ap(), KT[:].bitcast(F32))
                    nc.gpsimd.dma_start(dbg["vt"].ap(), VT[:])
                    nc.gpsimd.dma_start(dbg["vn"].ap(), Vn[:])
                    nc.gpsimd.dma_start(dbg["aot0"].ap(), AOT_t[0][:])

    nc.compile()
    return nc


_BUILD_CACHE = {}


def _get_kernel(causal: bool):
    if causal not in _BUILD_CACHE:
        _BUILD_CACHE[causal] = _build_kernel(causal)
    return _BUILD_CACHE[causal]


def _dequant_np(packed, scales, out_f, in_f):
    """Exact numpy port of the reference dequantize_q40."""
    w = np.asarray(packed)
    if w.dtype != np.int8:
        w = w.astype(np.int8)
    s = np.asarray(scales, dtype=np.float32).reshape(-1, 1)
    msb = w >> 4                                   # arithmetic, sign-extends
    lsb = (w << 4) >> 4                            # int8 wraps, then sign-extends
    grp = np.concatenate([msb, lsb], axis=1).reshape(-1, GROUP).astype(np.float32)
    return (grp * s).reshape(out_f, in_f)


def _canonical_causal_mask():
    causal = np.triu(np.ones((S, S), dtype=bool), k=1)
    return np.where(causal, np.float32(NEG), np.float32(0.0)).astype(np.float32)


def build_in_maps(inputs, causal):
    x = np.asarray(inputs["x"], dtype=np.float32)
    cos = np.asarray(inputs["cos"], dtype=np.float32)
    sin = np.asarray(inputs["sin"], dtype=np.float32)

    xT = np.ascontiguousarray(x.reshape(S, DIM).T).astype(ml_dtypes.bfloat16)
    cosT = np.ascontiguousarray(np.concatenate([cos.T, cos.T], axis=0))  # [128, S]
    sinT = np.ascontiguousarray(np.concatenate([sin.T, sin.T], axis=0))

    Wq = _dequant_np(inputs["wq"], inputs["sq"], N_HEADS * HEAD_DIM, DIM)
    Wk = _dequant_np(inputs["wk"], inputs["sk"], N_KV * HEAD_DIM, DIM)
    Wv = _dequant_np(inputs["wv"], inputs["sv"], N_KV * HEAD_DIM, DIM)
    Wo = _dequant_np(inputs["wo"], inputs["so"], DIM, N_HEADS * HEAD_DIM)

    in_maps = []
    for c in range(NCORES):
        q0 = c * QDIM_LOC
        k0 = c * HEAD_DIM
        wqkvT = np.empty((DIM, 768), dtype=ml_dtypes.bfloat16)
        wqkvT[:, 0:512] = Wq[q0:q0 + QDIM_LOC].T.astype(ml_dtypes.bfloat16)
        wqkvT[:, 512:640] = Wk[k0:k0 + HEAD_DIM].T.astype(ml_dtypes.bfloat16)
        wqkvT[:, 640:768] = Wv[k0:k0 + HEAD_DIM].T.astype(ml_dtypes.bfloat16)
        woT = np.ascontiguousarray(
            Wo[:, q0:q0 + QDIM_LOC].T).astype(ml_dtypes.bfloat16)  # [512, 4096]
        m = dict(xT=xT, wqkvT=wqkvT, woT=woT, cosT=cosT, sinT=sinT)
        if not causal:
            mask = np.asarray(inputs["mask"], dtype=np.float32)
            m["maskT"] = np.ascontiguousarray(mask.T)
        in_maps.append(m)
    return in_maps


def kernel(**inputs):
    mask = np.asarray(inputs["mask"], dtype=np.float32)
    causal = bool(np.array_equal(mask, _canonical_causal_mask()))
    nc = _get_kernel(causal)
    in_maps = build_in_maps(inputs, causal)
    res = bass_utils.run_bass_kernel_spmd(nc, in_maps, core_ids=list(range(NCORES)))
    acc = np.zeros((S, DIM), dtype=np.float32)
    for r in res.results:
        acc += np.asarray(r["out_partial"]).astype(np.float32)
    return acc.reshape(1, S, DIM)


if __name__ == "__main__":
    print("building causal kernel...")
    _get_kernel(True)
    print("built")
